# Initial kernel scaffold
#
"""AttentionBlock (GroupNorm + single-head full attention + residual) on 8
Trainium2 NeuronCores.

Sharding: data-parallel over batch (4) x sequence-parallel over query
tokens (2 halves of h*w=4096). Each core computes GroupNorm stats + K/V
for its whole batch and attention for its 2048-query half. No
collectives; host scatters inputs / gathers outputs.

Per-core layout: channels on partitions ([256 = 2x128p, n] tiles).
 - GN: per-channel mean/var via bn_stats, group reduce via tiny indicator
   matmuls on the PE, normalize folded as x*a+b (a,b per channel),
   written in-place as float32r.
 - qkv = W^T.T @ x_norm in f32r (full PE rate at N>=512), PSUM->SBUF
   copy adds bias.
 - Attention per 512-query chunk: S^T[k,q] = K_tile^T.T @ Q_chunk;
   P^T = exp(S^T/16) on ScalarE (scores ~ N(0,1): no max subtraction);
   out2[q,:] = sum_k P^T[:,q].T @ [V2^T | 1] where V2^T = V^T @ out_w^T,
   so the ones column accumulates the softmax denominator for free.
   Row-scale by 1/l, PE-transpose back to [c, q], add residual x + out_b.
"""

import numpy as np

B, C, HW = 4, 256, 4096
NQ = HW // 2
G = 8
CPG = C // G  # channels per group
EPS = 1e-5
N_CORES = 8

_CACHE = {}


def _build_nc(loop_reps=1):
    import bass_rust
    import concourse.bass as bass
    import concourse.mybir as mybir
    import concourse.tile as tile
    from concourse.masks import make_identity
    from concourse.vector_clock import ScopedClock

    F32 = mybir.dt.float32
    FR = mybir.dt.float32r
    AF = mybir.ActivationFunctionType
    ALU = mybir.AluOpType

    MAXW = 1

    class SplitWaitTileContext(tile.TileContext):
        """Workaround: this toolchain's walrus accepts at most one sync-wait
        per instruction; split excess waits onto same-engine InstNoOps."""

        def _split_excess_waits(self, inst):
            si = inst.sync_info
            if si is None:
                return []
            waits = list(si.on_wait)
            if len(waits) <= MAXW:
                return []
            extra, keep = waits[:-MAXW], waits[-MAXW:]
            nops = [
                mybir.InstNoOp(
                    name=f"I-{self.nc.next_id()}",
                    sync_info=mybir.SyncInfo(on_wait=[w], on_update=[]),
                    bass_nofuse=True,
                    engine=inst.engine,
                )
                for w in extra
            ]
            inst.sync_info = mybir.SyncInfo(on_wait=keep, on_update=list(si.on_update))
            return nops

        def _commit_and_lower(self, inst, original_block, old_bb_map, bb_to_exit_bb):
            for nop in self._split_excess_waits(inst):
                self._commit_instruction(nop, lazy_reg_writes=False)
            return super()._commit_and_lower(
                inst, original_block, old_bb_map, bb_to_exit_bb
            )

        def _drain_and_barrier(self, tick_clock, wait_clock):
            drain_inst = self.nc.sync.drain()
            wait_clock.add_sem_waits(
                drain_inst.ins, ScopedClock({None: tick_clock.global_clock})
            )
            si = drain_inst.ins.sync_info
            waits = list(si.on_wait) if si is not None else []
            if len(waits) > MAXW:
                updates = list(si.on_update) if si is not None else []
                drain_inst.ins.sync_info = bass_rust.SyncInfo(
                    on_wait=waits[:MAXW], on_update=[]
                )
                rest = waits[MAXW:]
                for i, w in enumerate(rest):
                    extra = self.nc.sync.drain()
                    extra.ins.sync_info = bass_rust.SyncInfo(
                        on_wait=[w], on_update=updates if i == len(rest) - 1 else []
                    )
            self.nc.all_engine_barrier()
            assert self.sems is not None
            popped = self.nc._tile_sem_poison_stack.pop()
            assert popped is self._sem_poison
            self.nc.clear_and_free_semaphores(list(self.sems.allocated().values()))
            self.nc.all_engine_barrier()

    nc = bass.Bass()
    xb = nc.dram_tensor("xb", [C, HW], F32, kind="ExternalInput")
    xq = nc.dram_tensor("xq", [C, NQ], F32, kind="ExternalInput")
    qkv_w = nc.dram_tensor("qkv_w", [3 * C, C], F32, kind="ExternalInput")
    qkv_b = nc.dram_tensor("qkv_b", [3 * C], F32, kind="ExternalInput")
    out_w = nc.dram_tensor("out_w", [C, C], F32, kind="ExternalInput")
    out_b = nc.dram_tensor("out_b", [C], F32, kind="ExternalInput")
    gn_gamma = nc.dram_tensor("gn_gamma", [C], F32, kind="ExternalInput")
    gn_beta = nc.dram_tensor("gn_beta", [C], F32, kind="ExternalInput")
    y = nc.dram_tensor("y", [C, NQ], F32, kind="ExternalOutput")

    with SplitWaitTileContext(nc) as tc:
        import contextlib

        ctx = contextlib.ExitStack()
        with ctx:
            singles = ctx.enter_context(tc.tile_pool(name="singles", bufs=1))
            xpool = ctx.enter_context(tc.tile_pool(name="xpool", bufs=2))
            xqpool = ctx.enter_context(tc.tile_pool(name="xqpool", bufs=2))
            qpool = ctx.enter_context(tc.tile_pool(name="qpool", bufs=2))
            kpool = ctx.enter_context(tc.tile_pool(name="kpool", bufs=2))
            vpool = ctx.enter_context(tc.tile_pool(name="vpool", bufs=2))
            ypool = ctx.enter_context(tc.tile_pool(name="ypool", bufs=2))
            wpool = ctx.enter_context(tc.tile_pool(name="wpool", bufs=2))
            wnat = ctx.enter_context(tc.tile_pool(name="wnat", bufs=2))
            ppool = ctx.enter_context(tc.tile_pool(name="ppool", bufs=3))
            opool = ctx.enter_context(tc.tile_pool(name="opool", bufs=3))
            small = ctx.enter_context(tc.tile_pool(name="small", bufs=4))
            stat = ctx.enter_context(tc.tile_pool(name="stat", bufs=2))
            psmm = ctx.enter_context(tc.tile_pool(name="psmm", bufs=4, space="PSUM"))
            psov = ctx.enter_context(tc.tile_pool(name="psov", bufs=4, space="PSUM"))

            def body(rep):
                # ---------- constants ----------
                ident = singles.tile([128, 128], F32, tag="ident")
                make_identity(nc, ident)
                g_sb = []
                be_sb = []
                for t in range(2):
                    gt = singles.tile([128, 1], F32, tag=f"g{t}")
                    nc.sync.dma_start(out=gt, in_=gn_gamma[t * 128 : (t + 1) * 128, None])
                    g_sb.append(gt)
                    bt = singles.tile([128, 1], F32, tag=f"be{t}")
                    nc.sync.dma_start(out=bt, in_=gn_beta[t * 128 : (t + 1) * 128, None])
                    be_sb.append(bt)
                qb_sb = singles.tile([128, 6], F32, tag="qb")
                nc.sync.dma_start(out=qb_sb, in_=qkv_b.rearrange("(m p) -> p m", p=128))
                ob_sb = singles.tile([128, 2], F32, tag="ob")
                nc.sync.dma_start(out=ob_sb, in_=out_b.rearrange("(t p) -> p t", p=128))
                eps_sb = singles.tile([8, 1], F32, tag="eps")
                nc.vector.memset(eps_sb, EPS)
                # group indicator for group-sum matmul: Gt[p, g]=1 iff chan in g
                gind = []
                hind = []
                for t in range(2):
                    gi = singles.tile([128, 8], F32, tag=f"gi{t}")
                    nc.gpsimd.memset(gi, 0.0)
                    for g in range(4):
                        nc.gpsimd.memset(
                            gi[g * CPG : (g + 1) * CPG, 4 * t + g : 4 * t + g + 1], 1.0
                        )
                    gind.append(gi)
                    hi = singles.tile([8, 128], F32, tag=f"hi{t}")
                    nc.gpsimd.memset(hi, 0.0)
                    for g in range(4):
                        nc.gpsimd.memset(
                            hi[4 * t + g : 4 * t + g + 1, g * CPG : (g + 1) * CPG], 1.0
                        )
                    hind.append(hi)

                # ---------- load x ----------
                x_sb = []
                for t in range(2):
                    xt = xpool.tile([128, HW], F32, tag="xv")
                    nc.sync.dma_start(out=xt, in_=xb[t * 128 : (t + 1) * 128, :])
                    x_sb.append(xt)
                xq_sb = []
                for t in range(2):
                    xqt = xqpool.tile([128, NQ], F32, tag="xq")
                    nc.sync.dma_start(out=xqt, in_=xq[t * 128 : (t + 1) * 128, :])
                    xq_sb.append(xqt)

                # ---------- GroupNorm stats ----------
                BNF = nc.vector.BN_STATS_FMAX  # 512
                nsub = HW // BNF
                mv = []
                for t in range(2):
                    st = stat.tile([128, nsub, nc.vector.BN_STATS_DIM], F32, tag="bst")
                    xr = x_sb[t].rearrange("p (s f) -> p s f", f=BNF)
                    for s in range(nsub):
                        nc.vector.bn_stats(out=st[:, s, :], in_=xr[:, s, :])
                    mvt = stat.tile([128, nc.vector.BN_AGGR_DIM], F32, tag=f"mv{t}")
                    nc.vector.bn_aggr(out=mvt, in_=st)
                    mv.append(mvt)
                # per-channel [mean, E[x^2]]
                st2 = []
                for t in range(2):
                    s2t = stat.tile([128, 2], F32, tag=f"st2{t}")
                    nc.vector.tensor_copy(s2t[:, 0:1], mv[t][:, 0:1])
                    # E[x^2] = var + mean^2
                    nc.vector.tensor_mul(s2t[:, 1:2], mv[t][:, 0:1], mv[t][:, 0:1])
                    nc.vector.tensor_add(s2t[:, 1:2], s2t[:, 1:2], mv[t][:, 1:2])
                    st2.append(s2t)
                psg = psmm.tile([8, 2], F32, tag="mm")
                nc.tensor.matmul(psg, gind[0], st2[0], start=True, stop=False)
                nc.tensor.matmul(psg, gind[1], st2[1], start=False, stop=True)
                gstat = stat.tile([8, 2], F32, tag="gstat")  # [mean_g, E[x^2]_g]
                nc.vector.tensor_scalar_mul(gstat, psg, 1.0 / CPG)
                var_g = stat.tile([8, 1], F32, tag="varg")
                nc.vector.tensor_mul(var_g, gstat[:, 0:1], gstat[:, 0:1])
                nc.vector.tensor_sub(var_g, gstat[:, 1:2], var_g)
                std_g = stat.tile([8, 1], F32, tag="stdg")
                nc.scalar.activation(out=std_g, in_=var_g, func=AF.Sqrt, bias=eps_sb, scale=1.0)
                rm = stat.tile([8, 2], F32, tag="rm")  # [rstd_g, mean_g]
                nc.vector.reciprocal(rm[:, 0:1], std_g)
                nc.vector.tensor_copy(rm[:, 1:2], gstat[:, 0:1])
                # broadcast to channels: [rstd_c, mean_c] = H_t.T @ rm
                ab = []
                for t in range(2):
                    psb = psmm.tile([128, 2], F32, tag="mm")
                    nc.tensor.matmul(psb, hind[t], rm, start=True, stop=True)
                    abt = stat.tile([128, 2], F32, tag=f"ab{t}")  # [a_c, b_c]
                    nc.vector.tensor_mul(abt[:, 0:1], psb[:, 0:1], g_sb[t])
                    nc.vector.tensor_mul(abt[:, 1:2], psb[:, 1:2], abt[:, 0:1])
                    nc.vector.tensor_sub(abt[:, 1:2], be_sb[t], abt[:, 1:2])
                    ab.append(abt)

                # ---------- residual prep (reads raw xq before overwrite) ----
                y_sb = []
                for t in range(2):
                    yt = ypool.tile([128, NQ], F32, tag="y")
                    nc.vector.tensor_scalar_add(yt, xq_sb[t], ob_sb[:, t : t + 1])
                    y_sb.append(yt)

                # ---------- normalize in place -> f32r ----------
                xn = []
                xqn = []
                for t in range(2):
                    nc.vector.tensor_scalar(
                        out=x_sb[t][:, :].bitcast(FR),
                        in0=x_sb[t],
                        scalar1=ab[t][:, 0:1],
                        scalar2=ab[t][:, 1:2],
                        op0=ALU.mult,
                        op1=ALU.add,
                    )
                    xn.append(x_sb[t][:, :].bitcast(FR))
                    nc.scalar.activation(
                        out=xq_sb[t][:, :].bitcast(FR),
                        in_=xq_sb[t],
                        func=AF.Identity,
                        bias=ab[t][:, 1:2],
                        scale=ab[t][:, 0:1],
                    )
                    xqn.append(xq_sb[t][:, :].bitcast(FR))

                # ---------- transpose weights ----------
                wT = []  # qkv_w^T tiles [c_in 128, 768] f32r
                for t in range(2):
                    wT.append(wpool.tile([128, 768], FR, tag=f"wT{t}"))
                owT = []  # out_w^T tiles [c_in 128, 256] f32r
                for t in range(2):
                    owT.append(wpool.tile([128, 256], FR, tag=f"owT{t}"))
                for i in range(6):
                    wn = wnat.tile([128, C], F32, tag="wn")
                    nc.sync.dma_start(out=wn, in_=qkv_w[i * 128 : (i + 1) * 128, :])
                    for t in range(2):
                        pst = psmm.tile([128, 128], F32, tag="mm")
                        nc.tensor.transpose(pst, wn[:, t * 128 : (t + 1) * 128], ident)
                        eng = nc.vector if (i + t) % 2 == 0 else nc.scalar
                        eng.tensor_copy(wT[t][:, i * 128 : (i + 1) * 128], pst)
                for i in range(2):
                    wn = wnat.tile([128, C], F32, tag="wn")
                    nc.sync.dma_start(out=wn, in_=out_w[i * 128 : (i + 1) * 128, :])
                    for t in range(2):
                        pst = psmm.tile([128, 128], F32, tag="mm")
                        nc.tensor.transpose(pst, wn[:, t * 128 : (t + 1) * 128], ident)
                        eng = nc.vector if (i + t) % 2 == 0 else nc.scalar
                        eng.tensor_copy(owT[i][:, t * 128 : (t + 1) * 128], pst)

                # ---------- qkv projections ----------
                q_sb = [qpool.tile([128, NQ], FR, tag=f"q{t}") for t in range(2)]
                k_sb = [kpool.tile([128, HW], FR, tag=f"k{t}") for t in range(2)]
                v_sb = [vpool.tile([128, HW], FR, tag="vy") for t in range(2)]
                nch = 0
                for m in range(6):
                    dst = (q_sb, k_sb, v_sb)[m // 2][m % 2]
                    src = xqn if m < 2 else xn
                    nj = NQ // 512 if m < 2 else HW // 512
                    for j in range(nj):
                        ps = psmm.tile([128, 512], F32, tag="mm")
                        nc.tensor.matmul(
                            ps,
                            wT[0][:, m * 128 : (m + 1) * 128],
                            src[0][:, j * 512 : (j + 1) * 512],
                            start=True,
                            stop=False,
                        )
                        nc.tensor.matmul(
                            ps,
                            wT[1][:, m * 128 : (m + 1) * 128],
                            src[1][:, j * 512 : (j + 1) * 512],
                            start=False,
                            stop=True,
                        )
                        dslice = dst[:, j * 512 : (j + 1) * 512]
                        if nch % 2 == 0:
                            nc.vector.tensor_scalar_add(dslice, ps, qb_sb[:, m : m + 1])
                        else:
                            nc.scalar.activation(
                                out=dslice, in_=ps, func=AF.Identity,
                                bias=qb_sb[:, m : m + 1], scale=1.0,
                            )
                        nch += 1

                # ---------- V2^T = V^T @ out_w^T (+ ones col) ----------
                v2t = []
                for h in range(2):
                    v2 = xpool.tile([128, 16, 257], FR, tag="xv")
                    nc.vector.memset(v2[:, :, 256:257], 1.0)
                    v2t.append(v2)
                for nt in range(32):
                    ps = psmm.tile([128, 256], F32, tag="mm")
                    nc.tensor.matmul(
                        ps, v_sb[0][:, nt * 128 : (nt + 1) * 128], owT[0],
                        start=True, stop=False,
                    )
                    nc.tensor.matmul(
                        ps, v_sb[1][:, nt * 128 : (nt + 1) * 128], owT[1],
                        start=False, stop=True,
                    )
                    dst = v2t[nt // 16][:, nt % 16, 0:256]
                    if nt % 2 == 0:
                        nc.vector.tensor_copy(dst, ps)
                    else:
                        nc.scalar.tensor_copy(dst, ps)

                # ---------- attention ----------
                for qc in range(NQ // 512):
                    po = [psov.tile([128, 257], F32, tag="o") for _ in range(4)]
                    for kt in range(32):
                        ps = psmm.tile([128, 512], F32, tag="mm")
                        nc.tensor.matmul(
                            ps, k_sb[0][:, kt * 128 : (kt + 1) * 128],
                            q_sb[0][:, qc * 512 : (qc + 1) * 512],
                            start=True, stop=False,
                        )
                        nc.tensor.matmul(
                            ps, k_sb[1][:, kt * 128 : (kt + 1) * 128],
                            q_sb[1][:, qc * 512 : (qc + 1) * 512],
                            start=False, stop=True,
                        )
                        pT = ppool.tile([128, 512], FR, tag="p")
                        nc.scalar.activation(
                            out=pT, in_=ps, func=AF.Exp, scale=1.0 / 16.0
                        )
                        for s in range(4):
                            nc.tensor.matmul(
                                po[s],
                                pT[:, s * 128 : (s + 1) * 128],
                                v2t[kt // 16][:, kt % 16, :],
                                start=(kt == 0),
                                stop=(kt == 31),
                                skip_group_check=True,
                            )
                    for s in range(4):
                        rl = small.tile([128, 1], F32, tag="rl")
                        nc.vector.reciprocal(rl, po[s][:, 256:257])
                        o_sb = opool.tile([128, 256], F32, tag="osb")
                        nc.vector.tensor_scalar_mul(o_sb, po[s][:, 0:256], rl)
                        for t in range(2):
                            pst = psmm.tile([128, 128], F32, tag="mm")
                            nc.tensor.transpose(
                                pst, o_sb[:, t * 128 : (t + 1) * 128], ident
                            )
                            ys = y_sb[t][:, qc * 512 + s * 128 : qc * 512 + (s + 1) * 128]
                            eng = nc.vector if (s + t) % 2 == 0 else nc.scalar
                            eng.tensor_tensor(ys, pst, ys, ALU.add)
                    for t in range(2):
                        nc.sync.dma_start(
                            out=y[t * 128 : (t + 1) * 128, qc * 512 : (qc + 1) * 512],
                            in_=y_sb[t][:, qc * 512 : (qc + 1) * 512],
                        )

            if loop_reps == 1:
                body(0)
            else:
                with tc.For_i(0, loop_reps, 1) as _i:
                    body(0)

    return nc


def _get_runner(loop_reps=1):
    key = ("runner", loop_reps)
    if key not in _CACHE:
        nc = _build_nc(loop_reps)
        _CACHE[key] = nc
    return _CACHE[key]


def kernel(x, gn_gamma, gn_beta, qkv_w, qkv_b, out_w, out_b):
    from concourse.bass_utils import run_bass_kernel_spmd

    x = np.asarray(x, dtype=np.float32)
    gn_gamma = np.asarray(gn_gamma, dtype=np.float32)
    gn_beta = np.asarray(gn_beta, dtype=np.float32)
    qkv_w = np.asarray(qkv_w, dtype=np.float32)
    qkv_b = np.asarray(qkv_b, dtype=np.float32)
    out_w = np.asarray(out_w, dtype=np.float32)
    out_b = np.asarray(out_b, dtype=np.float32)

    b, c, h, w = x.shape
    assert (b, c, h * w) == (B, C, HW)
    xf = x.reshape(b, c, HW)

    nc = _get_runner()
    in_maps = []
    for j in range(N_CORES):
        bi, qh = j // 2, j % 2
        in_maps.append(
            {
                "xb": np.ascontiguousarray(xf[bi]),
                "xq": np.ascontiguousarray(xf[bi][:, qh * NQ : (qh + 1) * NQ]),
                "qkv_w": qkv_w,
                "qkv_b": qkv_b,
                "out_w": out_w,
                "out_b": out_b,
                "gn_gamma": gn_gamma,
                "gn_beta": gn_beta,
            }
        )
    res = run_bass_kernel_spmd(nc, in_maps, core_ids=list(range(N_CORES)))
    out = np.empty((B, C, HW), dtype=np.float32)
    for j in range(N_CORES):
        bi, qh = j // 2, j % 2
        out[bi][:, qh * NQ : (qh + 1) * NQ] = res.results[j]["y"]
    return out.reshape(b, c, h, w)


# revision 16
# speedup vs baseline: 1.1592x; 1.1592x over previous
"""AttentionBlock (GroupNorm + single-head full attention + residual) on 8
Trainium2 NeuronCores.

Sharding: data-parallel over batch (4) x sequence-parallel over query
tokens (2 halves of h*w=4096). Each core computes GroupNorm stats + K/V
for its whole batch and attention for its 2048-query half. No
collectives; host scatters inputs / gathers outputs.

Per-core layout: channels on partitions ([256 = 2x128p, n] tiles).
 - GN: per-channel mean/var via bn_stats, group reduce via tiny indicator
   matmuls on the PE, normalize folded as x*a+b (a,b per channel),
   written in-place as float32r.
 - qkv = W^T.T @ x_norm in f32r (full PE rate at N>=512), PSUM->SBUF
   copy adds bias.
 - Attention per 512-query chunk: S^T[k,q] = K_tile^T.T @ Q_chunk;
   P^T = exp(S^T/16) on ScalarE (scores ~ N(0,1): no max subtraction);
   out2[q,:] = sum_k P^T[:,q].T @ [V2^T | 1] where V2^T = V^T @ out_w^T,
   so the ones column accumulates the softmax denominator for free.
   Row-scale by 1/l, PE-transpose back to [c, q], add residual x + out_b.
"""

import numpy as np

B, C, HW = 4, 256, 4096
NQ = HW // 2
G = 8
CPG = C // G  # channels per group
EPS = 1e-5
N_CORES = 8

_CACHE = {}


def _build_nc(loop_reps=1, debug=False):
    import bass_rust
    import concourse.bass as bass
    import concourse.mybir as mybir
    import concourse.tile as tile
    from concourse.masks import make_identity
    from concourse.vector_clock import ScopedClock

    F32 = mybir.dt.float32
    FR = mybir.dt.float32r
    AF = mybir.ActivationFunctionType
    ALU = mybir.AluOpType

    MAXW = 1

    class SplitWaitTileContext(tile.TileContext):
        """Workaround: this toolchain's walrus accepts at most one sync-wait
        per instruction; split excess waits onto same-engine InstNoOps."""

        def _split_excess_waits(self, inst):
            si = inst.sync_info
            if si is None:
                return []
            waits = list(si.on_wait)
            if len(waits) <= MAXW:
                return []
            extra, keep = waits[:-MAXW], waits[-MAXW:]
            nops = [
                mybir.InstNoOp(
                    name=f"I-{self.nc.next_id()}",
                    sync_info=mybir.SyncInfo(on_wait=[w], on_update=[]),
                    bass_nofuse=True,
                    engine=inst.engine,
                )
                for w in extra
            ]
            inst.sync_info = mybir.SyncInfo(on_wait=keep, on_update=list(si.on_update))
            return nops

        def _commit_and_lower(self, inst, original_block, old_bb_map, bb_to_exit_bb):
            for nop in self._split_excess_waits(inst):
                self._commit_instruction(nop, lazy_reg_writes=False)
            return super()._commit_and_lower(
                inst, original_block, old_bb_map, bb_to_exit_bb
            )

        def _drain_and_barrier(self, tick_clock, wait_clock):
            drain_inst = self.nc.sync.drain()
            wait_clock.add_sem_waits(
                drain_inst.ins, ScopedClock({None: tick_clock.global_clock})
            )
            si = drain_inst.ins.sync_info
            waits = list(si.on_wait) if si is not None else []
            if len(waits) > MAXW:
                updates = list(si.on_update) if si is not None else []
                drain_inst.ins.sync_info = bass_rust.SyncInfo(
                    on_wait=waits[:MAXW], on_update=[]
                )
                rest = waits[MAXW:]
                for i, w in enumerate(rest):
                    extra = self.nc.sync.drain()
                    extra.ins.sync_info = bass_rust.SyncInfo(
                        on_wait=[w], on_update=updates if i == len(rest) - 1 else []
                    )
            self.nc.all_engine_barrier()
            assert self.sems is not None
            popped = self.nc._tile_sem_poison_stack.pop()
            assert popped is self._sem_poison
            self.nc.clear_and_free_semaphores(list(self.sems.allocated().values()))
            self.nc.all_engine_barrier()

    nc = bass.Bass()
    xb = nc.dram_tensor("xb", [C, HW], F32, kind="ExternalInput")
    xq = nc.dram_tensor("xq", [C, NQ], F32, kind="ExternalInput")
    qkv_w = nc.dram_tensor("qkv_w", [3 * C, C], F32, kind="ExternalInput")
    qkv_b = nc.dram_tensor("qkv_b", [3 * C], F32, kind="ExternalInput")
    out_w = nc.dram_tensor("out_w", [C, C], F32, kind="ExternalInput")
    out_b = nc.dram_tensor("out_b", [C], F32, kind="ExternalInput")
    gn_gamma = nc.dram_tensor("gn_gamma", [C], F32, kind="ExternalInput")
    gn_beta = nc.dram_tensor("gn_beta", [C], F32, kind="ExternalInput")
    gind_in = nc.dram_tensor("gind_in", [128, 16], F32, kind="ExternalInput")
    hind_in = nc.dram_tensor("hind_in", [8, 128 * 2], F32, kind="ExternalInput")
    ones_in = nc.dram_tensor("ones_in", [128, 32], F32, kind="ExternalInput")
    y = nc.dram_tensor("y", [C, NQ], F32, kind="ExternalOutput")
    if debug:
        d_xn = nc.dram_tensor("d_xn", [C, HW], F32, kind="ExternalOutput")
        d_q = nc.dram_tensor("d_q", [C, NQ], F32, kind="ExternalOutput")
        d_k = nc.dram_tensor("d_k", [C, HW], F32, kind="ExternalOutput")
        d_v2t = nc.dram_tensor("d_v2t", [HW, 258], F32, kind="ExternalOutput")
        d_po = nc.dram_tensor("d_po", [128, 258], F32, kind="ExternalOutput")
        d_ab = nc.dram_tensor("d_ab", [C, 2], F32, kind="ExternalOutput")

    with SplitWaitTileContext(nc) as tc:
        import contextlib

        ctx = contextlib.ExitStack()
        with ctx:
            singles = ctx.enter_context(tc.tile_pool(name="singles", bufs=1))
            xpool = ctx.enter_context(tc.tile_pool(name="xpool", bufs=2))
            xqpool = ctx.enter_context(tc.tile_pool(name="xqpool", bufs=2))
            qpool = ctx.enter_context(tc.tile_pool(name="qpool", bufs=2))
            kpool = ctx.enter_context(tc.tile_pool(name="kpool", bufs=2))
            vpool = ctx.enter_context(tc.tile_pool(name="vpool", bufs=2))
            ypool = ctx.enter_context(tc.tile_pool(name="ypool", bufs=2))
            wpool = ctx.enter_context(tc.tile_pool(name="wpool", bufs=1))
            wnat = ctx.enter_context(tc.tile_pool(name="wnat", bufs=2))
            ppool = ctx.enter_context(tc.tile_pool(name="ppool", bufs=3))
            opool = ctx.enter_context(tc.tile_pool(name="opool", bufs=3))
            small = ctx.enter_context(tc.tile_pool(name="small", bufs=4))
            stat = ctx.enter_context(tc.tile_pool(name="stat", bufs=2))
            psmm = ctx.enter_context(tc.tile_pool(name="psmm", bufs=4, space="PSUM"))
            psov = ctx.enter_context(tc.tile_pool(name="psov", bufs=4, space="PSUM"))

            def body(rep):
                # ---------- constants ----------
                ident = singles.tile([128, 128], F32, tag="ident")
                make_identity(nc, ident)
                gam_sb = singles.tile([128, 2], F32, tag="gam")
                nc.sync.dma_start(out=gam_sb, in_=gn_gamma.rearrange("(t p) -> p t", p=128))
                bet_sb = singles.tile([128, 2], F32, tag="bet")
                nc.sync.dma_start(out=bet_sb, in_=gn_beta.rearrange("(t p) -> p t", p=128))
                g_sb = [gam_sb[:, t : t + 1] for t in range(2)]
                be_sb = [bet_sb[:, t : t + 1] for t in range(2)]
                qb_sb = singles.tile([128, 6], F32, tag="qb")
                nc.sync.dma_start(out=qb_sb, in_=qkv_b.rearrange("(m p) -> p m", p=128))
                ob_sb = singles.tile([128, 2], F32, tag="ob")
                nc.sync.dma_start(out=ob_sb, in_=out_b.rearrange("(t p) -> p t", p=128))
                eps_sb = singles.tile([8, 1], F32, tag="eps")
                nc.vector.memset(eps_sb, EPS)
                # group indicators (host-built): gind[t][p, g]=1 iff channel
                # t*128+p is in group g; hind[t][g, p] likewise transposed.
                gi_sb = singles.tile([128, 16], F32, tag="gi")
                nc.sync.dma_start(out=gi_sb, in_=gind_in[:, :])
                gind = [gi_sb[:, 0:8], gi_sb[:, 8:16]]
                hi_sb = singles.tile([8, 128 * 2], F32, tag="hi")
                nc.sync.dma_start(out=hi_sb, in_=hind_in[:, :])
                hind = [hi_sb[:, 0:128], hi_sb[:, 128:256]]

                # ---------- load x / xq straight into f32r tiles ----------
                x_sb = []
                for t in range(2):
                    xt = xpool.tile([128, HW], FR, tag="xv", name=f"x{t}")
                    nc.gpsimd.dma_start(out=xt, in_=xb[t * 128 : (t + 1) * 128, :])
                    x_sb.append(xt)
                xq_sb = []
                for t in range(2):
                    xqt = xqpool.tile([128, NQ], FR, tag="xq", name=f"xq{t}")
                    nc.gpsimd.dma_start(out=xqt, in_=xq[t * 128 : (t + 1) * 128, :])
                    xq_sb.append(xqt)

                # ---------- GroupNorm stats ----------
                BNF = nc.vector.BN_STATS_FMAX  # 512
                nsub = HW // BNF
                mv = []
                for t in range(2):
                    st = stat.tile([128, nsub, nc.vector.BN_STATS_DIM], F32, tag="bst")
                    xr = x_sb[t][:, :].bitcast(F32).rearrange("p (s f) -> p s f", f=BNF)
                    for s in range(nsub):
                        nc.vector.bn_stats(out=st[:, s, :], in_=xr[:, s, :])
                    mvt = stat.tile([128, nc.vector.BN_AGGR_DIM], F32, tag=f"mv{t}")
                    nc.vector.bn_aggr(out=mvt, in_=st)
                    mv.append(mvt)
                # per-channel [mean, E[x^2]]
                st2 = []
                for t in range(2):
                    s2t = stat.tile([128, 2], F32, tag=f"st2{t}")
                    nc.vector.tensor_copy(s2t[:, 0:1], mv[t][:, 0:1])
                    # E[x^2] = var + mean^2
                    nc.vector.tensor_mul(s2t[:, 1:2], mv[t][:, 0:1], mv[t][:, 0:1])
                    nc.vector.tensor_add(s2t[:, 1:2], s2t[:, 1:2], mv[t][:, 1:2])
                    st2.append(s2t)
                psg = psmm.tile([8, 2], F32, tag="mm")
                nc.tensor.matmul(psg, gind[0], st2[0], start=True, stop=False)
                nc.tensor.matmul(psg, gind[1], st2[1], start=False, stop=True)
                gstat = stat.tile([8, 2], F32, tag="gstat")  # [mean_g, E[x^2]_g]
                nc.vector.tensor_scalar_mul(gstat, psg, 1.0 / CPG)
                var_g = stat.tile([8, 1], F32, tag="varg")
                nc.vector.tensor_mul(var_g, gstat[:, 0:1], gstat[:, 0:1])
                nc.vector.tensor_sub(var_g, gstat[:, 1:2], var_g)
                std_g = stat.tile([8, 1], F32, tag="stdg")
                nc.scalar.activation(out=std_g, in_=var_g, func=AF.Sqrt, bias=eps_sb, scale=1.0)
                rm = stat.tile([8, 2], F32, tag="rm")  # [rstd_g, mean_g]
                nc.vector.reciprocal(rm[:, 0:1], std_g)
                nc.vector.tensor_copy(rm[:, 1:2], gstat[:, 0:1])
                # broadcast to channels: [rstd_c, mean_c] = H_t.T @ rm
                ab = []
                for t in range(2):
                    psb = psmm.tile([128, 2], F32, tag="mm")
                    nc.tensor.matmul(psb, hind[t], rm, start=True, stop=True)
                    abt = stat.tile([128, 2], F32, tag=f"ab{t}")  # [a_c, b_c]
                    nc.vector.tensor_mul(abt[:, 0:1], psb[:, 0:1], g_sb[t])
                    nc.vector.tensor_mul(abt[:, 1:2], psb[:, 1:2], abt[:, 0:1])
                    nc.vector.tensor_sub(abt[:, 1:2], be_sb[t], abt[:, 1:2])
                    ab.append(abt)

                # ---------- residual prep (reads raw xq before overwrite) ----
                y_sb = []
                for t in range(2):
                    yt = ypool.tile([128, NQ], F32, tag="y")
                    nc.vector.tensor_scalar_add(yt, xq_sb[t][:, :].bitcast(F32), ob_sb[:, t : t + 1])
                    y_sb.append(yt)

                # ---------- normalize in place -> f32r ----------
                xn = []
                xqn = []
                for t in range(2):
                    nc.vector.tensor_scalar(
                        out=x_sb[t][:, :],
                        in0=x_sb[t][:, :].bitcast(F32),
                        scalar1=ab[t][:, 0:1],
                        scalar2=ab[t][:, 1:2],
                        op0=ALU.mult,
                        op1=ALU.add,
                    )
                    xn.append(x_sb[t])
                    nc.scalar.activation(
                        out=xq_sb[t][:, :],
                        in_=xq_sb[t][:, :].bitcast(F32),
                        func=AF.Identity,
                        bias=ab[t][:, 1:2],
                        scale=ab[t][:, 0:1],
                    )
                    xqn.append(xq_sb[t])

                # ---------- transpose weights ----------
                wT = []  # qkv_w^T tiles [c_in 128, 768] f32r
                for t in range(2):
                    wT.append(wpool.tile([128, 768], FR, tag=f"wT{t}", name=f"wTn{t}"))
                owT = []  # out_w^T tiles [c_in 128, 256] f32r
                for t in range(2):
                    owT.append(wpool.tile([128, 256], FR, tag=f"owT{t}", name=f"owT{t}"))
                for i in range(6):
                    wn = wnat.tile([128, C], F32, tag="wn")
                    nc.sync.dma_start(out=wn, in_=qkv_w[i * 128 : (i + 1) * 128, :])
                    for t in range(2):
                        pst = psmm.tile([128, 128], F32, tag="mm")
                        nc.tensor.transpose(pst, wn[:, t * 128 : (t + 1) * 128], ident)
                        if (i + t) % 2 == 0:
                            nc.vector.tensor_copy(wT[t][:, i * 128 : (i + 1) * 128], pst)
                        else:
                            nc.scalar.copy(wT[t][:, i * 128 : (i + 1) * 128], pst)
                for i in range(2):
                    wn = wnat.tile([128, C], F32, tag="wn")
                    nc.sync.dma_start(out=wn, in_=out_w[i * 128 : (i + 1) * 128, :])
                    for t in range(2):
                        pst = psmm.tile([128, 128], F32, tag="mm")
                        nc.tensor.transpose(pst, wn[:, t * 128 : (t + 1) * 128], ident)
                        if (i + t) % 2 == 0:
                            nc.vector.tensor_copy(owT[t][:, i * 128 : (i + 1) * 128], pst)
                        else:
                            nc.scalar.copy(owT[t][:, i * 128 : (i + 1) * 128], pst)

                # ---------- qkv projections ----------
                q_sb = [qpool.tile([128, NQ], FR, tag="q", name=f"q{t}") for t in range(2)]
                k_sb = [kpool.tile([128, HW], FR, tag="k", name=f"k{t}") for t in range(2)]
                v_sb = [vpool.tile([128, HW], FR, tag="vy", name=f"v{t}") for t in range(2)]
                nch = 0
                for m in range(6):
                    dst = (q_sb, k_sb, v_sb)[m // 2][m % 2]
                    src = xqn if m < 2 else xn
                    nj = NQ // 512 if m < 2 else HW // 512
                    for j in range(nj):
                        ps = psmm.tile([128, 512], F32, tag="mm")
                        nc.tensor.matmul(
                            ps,
                            wT[0][:, m * 128 : (m + 1) * 128],
                            src[0][:, j * 512 : (j + 1) * 512],
                            start=True,
                            stop=False,
                        )
                        nc.tensor.matmul(
                            ps,
                            wT[1][:, m * 128 : (m + 1) * 128],
                            src[1][:, j * 512 : (j + 1) * 512],
                            start=False,
                            stop=True,
                        )
                        dslice = dst[:, j * 512 : (j + 1) * 512]
                        if nch % 2 == 0:
                            nc.vector.tensor_scalar_add(dslice, ps, qb_sb[:, m : m + 1])
                        else:
                            nc.scalar.activation(
                                out=dslice, in_=ps, func=AF.Identity,
                                bias=qb_sb[:, m : m + 1], scale=1.0,
                            )
                        nch += 1

                # ---------- V2^T = V^T @ out_w^T (+ ones col) ----------
                v2t = []
                for h in range(2):
                    v2 = xpool.tile([128, 16, 258], FR, tag="xv", name=f"v2t{h}")
                    nc.gpsimd.dma_start(
                        out=v2[:, :, 256:258],
                        in_=ones_in.rearrange("p (f o) -> p f o", o=2),
                    )
                    v2t.append(v2)
                for nt in range(32):
                    ps = psmm.tile([128, 256], F32, tag="mm")
                    nc.tensor.matmul(
                        ps, v_sb[0][:, nt * 128 : (nt + 1) * 128], owT[0],
                        start=True, stop=False,
                    )
                    nc.tensor.matmul(
                        ps, v_sb[1][:, nt * 128 : (nt + 1) * 128], owT[1],
                        start=False, stop=True,
                    )
                    dst = v2t[nt // 16][:, nt % 16, 0:256]
                    if nt % 2 == 0:
                        nc.vector.tensor_copy(dst, ps)
                    else:
                        nc.scalar.copy(dst, ps)

                if debug:
                    for t in range(2):
                        nc.sync.dma_start(
                            out=d_xn[t * 128 : (t + 1) * 128, :],
                            in_=xn[t][:, :].bitcast(F32),
                        )
                        nc.sync.dma_start(
                            out=d_q[t * 128 : (t + 1) * 128, :],
                            in_=q_sb[t][:, :].bitcast(F32),
                        )
                        nc.sync.dma_start(
                            out=d_k[t * 128 : (t + 1) * 128, :],
                            in_=k_sb[t][:, :].bitcast(F32),
                        )
                        nc.sync.dma_start(
                            out=d_ab[t * 128 : (t + 1) * 128, :], in_=ab[t]
                        )
                    for h in range(2):
                        nc.sync.dma_start(
                            out=d_v2t.rearrange("(h f p) o -> h p f o", h=2, p=128)[h],
                            in_=v2t[h][:, :, :].bitcast(F32),
                        )

                # ---------- attention ----------
                for qc in range(NQ // 512):
                    po = [psov.tile([128, 258], F32, tag="o", name=f"po{s_}") for s_ in range(4)]
                    for kt in range(32):
                        ps = psmm.tile([128, 512], F32, tag="mm")
                        nc.tensor.matmul(
                            ps, k_sb[0][:, kt * 128 : (kt + 1) * 128],
                            q_sb[0][:, qc * 512 : (qc + 1) * 512],
                            start=True, stop=False,
                        )
                        nc.tensor.matmul(
                            ps, k_sb[1][:, kt * 128 : (kt + 1) * 128],
                            q_sb[1][:, qc * 512 : (qc + 1) * 512],
                            start=False, stop=True,
                        )
                        pT = ppool.tile([128, 512], FR, tag="p")
                        nc.scalar.activation(
                            out=pT, in_=ps, func=AF.Exp, scale=1.0 / 16.0
                        )
                        for s in range(4):
                            nc.tensor.matmul(
                                po[s],
                                pT[:, s * 128 : (s + 1) * 128],
                                v2t[kt // 16][:, kt % 16, :],
                                start=(kt == 0),
                                stop=(kt == 31),
                                skip_group_check=True,
                            )
                    if debug and qc == 0:
                        dpo = opool.tile([128, 258], F32, tag="dpo", name="dpo")
                        nc.vector.tensor_copy(dpo, po[0])
                        nc.sync.dma_start(out=d_po[:, :], in_=dpo)
                    for s in range(4):
                        rl = small.tile([128, 1], F32, tag="rl")
                        nc.vector.reciprocal(rl, po[s][:, 256:257])
                        o_sb = opool.tile([128, 256], F32, tag="osb")
                        nc.vector.tensor_scalar_mul(o_sb, po[s][:, 0:256], rl)
                        for t in range(2):
                            pst = psmm.tile([128, 128], F32, tag="mm")
                            nc.tensor.transpose(
                                pst, o_sb[:, t * 128 : (t + 1) * 128], ident
                            )
                            ys = y_sb[t][:, qc * 512 + s * 128 : qc * 512 + (s + 1) * 128]
                            nc.vector.tensor_tensor(ys, pst, ys, ALU.add)
                    for t in range(2):
                        nc.sync.dma_start(
                            out=y[t * 128 : (t + 1) * 128, qc * 512 : (qc + 1) * 512],
                            in_=y_sb[t][:, qc * 512 : (qc + 1) * 512],
                        )

            for rep in range(loop_reps):
                body(rep)

    return nc


def _get_runner(loop_reps=1):
    key = ("runner", loop_reps)
    if key not in _CACHE:
        nc = _build_nc(loop_reps)
        _CACHE[key] = nc
    return _CACHE[key]


def kernel(x, gn_gamma, gn_beta, qkv_w, qkv_b, out_w, out_b):
    from concourse.bass_utils import run_bass_kernel_spmd

    x = np.asarray(x, dtype=np.float32)
    gn_gamma = np.asarray(gn_gamma, dtype=np.float32)
    gn_beta = np.asarray(gn_beta, dtype=np.float32)
    qkv_w = np.asarray(qkv_w, dtype=np.float32)
    qkv_b = np.asarray(qkv_b, dtype=np.float32)
    out_w = np.asarray(out_w, dtype=np.float32)
    out_b = np.asarray(out_b, dtype=np.float32)

    b, c, h, w = x.shape
    assert (b, c, h * w) == (B, C, HW)
    xf = x.reshape(b, c, HW)

    nc = _get_runner()
    in_maps = []
    for j in range(N_CORES):
        bi, qh = j // 2, j % 2
        in_maps.append(
            {
                "xb": np.ascontiguousarray(xf[bi]),
                "xq": np.ascontiguousarray(xf[bi][:, qh * NQ : (qh + 1) * NQ]),
                "qkv_w": qkv_w,
                "qkv_b": qkv_b,
                "out_w": out_w,
                "out_b": out_b,
                "gn_gamma": gn_gamma,
                "gn_beta": gn_beta,
            }
        )
    gind = np.zeros((128, 16), dtype=np.float32)
    hind = np.zeros((8, 256), dtype=np.float32)
    for t in range(2):
        for p in range(128):
            g = (t * 128 + p) // CPG
            gind[p, t * 8 + g] = 1.0
            hind[g, t * 128 + p] = 1.0
    for m in in_maps:
        m["gind_in"] = gind
        m["hind_in"] = hind
        m["ones_in"] = np.tile(np.array([1.0, 0.0], dtype=np.float32), (128, 16))
    res = run_bass_kernel_spmd(nc, in_maps, core_ids=list(range(N_CORES)))
    out = np.empty((B, C, HW), dtype=np.float32)
    for j in range(N_CORES):
        bi, qh = j // 2, j % 2
        out[bi][:, qh * NQ : (qh + 1) * NQ] = res.results[j]["y"]
    return out.reshape(b, c, h, w)
